# revision 1
# baseline (speedup 1.0000x reference)
"""BitConv1d Trainium2 kernel.

Computes, for x:(8,512,8192) f32, weight:(512,512,7) f32, gamma:(512,) f32:
  rms  = sqrt(mean(x^2, channel) + 1e-6)          (per b,t)
  xn   = x / rms * gamma
  s    = max(|xn|) over the FULL batch  (clamped to >= 1e-5)
  q    = round(clip(xn/s*127, -128, 127))         (8-bit act quant, STE forward)
  ws   = max(mean(|w|), 1e-5); wq = round(clip(w/ws, -1, 1))  (ternary weights)
  out  = conv1d(q * s/127, wq, pad 3) * ws

Strategy: data-parallel over batch across 8 NeuronCores (1 batch element per
core), weights replicated. The activation-quant global max uses an on-device
AllReduce(max) of max(xn^2). The conv runs as 28 shifted bf16 matmuls per
output tile with exact integer arithmetic (q in [-127,127] and wq in {-1,0,1}
are exact in bf16; f32 PSUM accumulation of integers < 2^24 is exact), so the
conv result equals the integer conv scaled by s*ws/127. Rounding uses the
(x + 1.5*2^23) - 1.5*2^23 trick (round-half-even, matching jnp.round).
"""

import sys

sys.path.insert(0, "/opt/trn_rl_repo")

import numpy as np

N_CORES = 8
B, C, T = 8, 512, 8192
CO, K = 512, 7
CI_CHUNKS = 4  # 512 in-channels / 128 partitions
CB_BLOCKS = 4  # 512 out-channels / 128 partitions
TT = 512  # time-tile (columns per matmul)
PAD = 3  # conv padding

EPS_NORM = 1e-6
EPS_SCALE = 1e-5
QP = 127.0
C_MAGIC = 12582912.0  # 1.5 * 2^23 : (x + C) - C == round-half-even(x)
W_COUNT = CO * C * K

_CACHE = {}


def _build(n_cores: int, t_len: int):
    import contextlib
    import os
    skip_conv = os.environ.get("BITCONV_SKIP_CONV") == "1"
    skip_phase1 = os.environ.get("BITCONV_SKIP_PHASE1") == "1"
    skip_quant = os.environ.get("BITCONV_SKIP_QUANT") == "1"
    skip_w = os.environ.get("BITCONV_SKIP_W") == "1"
    skip_1a = os.environ.get("BITCONV_SKIP_1A") == "1"

    import concourse.bacc as bacc
    import concourse.bass as bass
    import concourse.tile as tile
    from concourse import bass_isa, mybir

    f32 = mybir.dt.float32
    bf16 = mybir.dt.bfloat16
    Alu = mybir.AluOpType
    Act = mybir.ActivationFunctionType
    ts = bass.ts

    NT = t_len // TT  # time tiles
    WQ_F = CB_BLOCKS * K * CI_CHUNKS * 128  # 14336
    NW = 16  # weight streaming chunks
    WCH = WQ_F // NW  # 896 columns per chunk

    nc = bacc.Bacc("TRN2", target_bir_lowering=False, debug=False,
                   num_devices=n_cores)

    x_t = nc.dram_tensor("x", [C, t_len], f32, kind="ExternalInput")
    wt_t = nc.dram_tensor("wt", [128, WQ_F], f32, kind="ExternalInput")
    g_t = nc.dram_tensor("g", [C], f32, kind="ExternalInput")
    out_t = nc.dram_tensor("out", [CO, t_len], f32, kind="ExternalOutput")

    xv = x_t[:].rearrange("(c p) t -> p c t", p=128)  # chunk-major channels

    with tile.TileContext(nc) as tc:
        with contextlib.ExitStack() as stk:
            singles = stk.enter_context(tc.tile_pool(name="singles", bufs=1))
            scr = stk.enter_context(tc.tile_pool(name="scr", bufs=5))
            bncp = stk.enter_context(tc.tile_pool(name="bncp", bufs=2))
            rmathp = stk.enter_context(tc.tile_pool(name="rmathp", bufs=5))
            scp = stk.enter_context(tc.tile_pool(name="scp", bufs=14))
            amaxp = stk.enter_context(tc.tile_pool(name="amaxp", bufs=2))
            rowp = stk.enter_context(tc.tile_pool(name="rowp", bufs=1))
            wstga = stk.enter_context(tc.tile_pool(name="wstga", bufs=2))
            dramp = stk.enter_context(
                tc.tile_pool(name="dram", bufs=1, space="DRAM"))
            ps_small = stk.enter_context(
                tc.tile_pool(name="ps_small", bufs=2, space="PSUM"))
            ps_mb = stk.enter_context(
                tc.tile_pool(name="ps_mb", bufs=2, space="PSUM"))
            ps_conv = stk.enter_context(
                tc.tile_pool(name="ps_conv", bufs=4, space="PSUM"))

            ones_col = singles.tile([128, 1], f32)
            nc.vector.memset(ones_col[:], 1.0)
            eps_col = singles.tile([128, 1], f32)
            nc.vector.memset(eps_col[:], EPS_NORM)
            zero_col = singles.tile([128, 1], f32)
            nc.vector.memset(zero_col[:], 0.0)
            g_row = singles.tile([1, C], f32)
            nc.sync.dma_start(g_row[:], g_t[:].rearrange("(a d) -> a d", a=1))

            cc_in = dramp.tile([128], f32)
            cc_out = dramp.tile([128], f32)

            FW = t_len // 128  # per-t arrays reshaped to (128, FW)
            PPT = TT // FW  # partitions covered by one t-tile

            # ---- phase 1: grouped pipeline: ssq -> r -> |xn| max ----------
            # r is per-timestep: compute it per group of 4 t-tiles and
            # overlap the max pass (1b) of group G with the streaming (1a)
            # of group G+1.
            with tc.tile_pool(name="xres", bufs=1) as xres:
                x_sb = xres.tile([128, CI_CHUNKS, t_len], f32)
                rcol = singles.tile([128, FW], f32)  # channel-summed x^2
                mcol = rmathp.tile([128, FW], f32, tag="rmath")
                s0 = rmathp.tile([128, FW], f32, tag="rmath")
                tdiv = rmathp.tile([128, FW], f32, tag="rmath")
                rhalf = rmathp.tile([128, FW], f32, tag="rmath")
                g2_row = singles.tile([1, C], f32)
                nc.vector.tensor_scalar_mul(g2_row[:], g_row[:], 2.0)
                r_row = rowp.tile([1, t_len], f32, tag="trow")
                coll = singles.tile([128, NT * CI_CHUNKS], f32)
                nc.vector.memset(coll[:], 0.0)

                GRP = min(4, NT)  # t-tiles per pipeline group
                NG = NT // GRP
                PG = PPT * GRP  # rcol partitions per group
                for G in range(NG):
                    for j in range(G * GRP, (G + 1) * GRP):
                        nc.sync.dma_start(x_sb[:, :, ts(j, TT)],
                                          xv[:, :, ts(j, TT)])
                        if skip_1a:
                            continue
                        ssq = ps_small.tile([1, TT], f32, tag="ssq")
                        for ci in range(CI_CHUNKS):
                            x2 = scr.tile([128, TT], f32, tag="scr")
                            nc.scalar.activation(x2[:], x_sb[:, ci, ts(j, TT)],
                                                 Act.Square)
                            nc.tensor.matmul(ssq[:], ones_col[:], x2[:],
                                             start=(ci == 0),
                                             stop=(ci == CI_CHUNKS - 1))
                        sbounce = bncp.tile([1, TT], f32, tag="sbounce")
                        nc.scalar.copy(sbounce[:], ssq[:])
                        nc.sync.dma_start(rcol[PPT * j:PPT * (j + 1), :],
                                          sbounce[:])
                    # r math for this group (1/(2*rms), Newton-refined sqrt)
                    gs = slice(PG * G, PG * (G + 1))
                    nc.vector.tensor_scalar(mcol[gs, :], rcol[gs, :], 1.0 / C,
                                            EPS_NORM, op0=Alu.mult, op1=Alu.add)
                    nc.scalar.activation(s0[gs, :], rcol[gs, :], Act.Sqrt,
                                         bias=eps_col[gs, :], scale=1.0 / C)
                    nc.vector.reciprocal(tdiv[gs, :], s0[gs, :])
                    nc.vector.tensor_tensor(tdiv[gs, :], mcol[gs, :],
                                            tdiv[gs, :], op=Alu.mult)
                    nc.vector.tensor_tensor(tdiv[gs, :], tdiv[gs, :],
                                            s0[gs, :], op=Alu.add)
                    nc.vector.reciprocal(rhalf[gs, :], tdiv[gs, :])
                    nc.sync.dma_start(
                        r_row[0:1, GRP * TT * G:GRP * TT * (G + 1)],
                        rhalf[gs, :])
                    # 1b for this group: local max of |xn|
                    for j in range(G * GRP, (G + 1) * GRP):
                        if skip_phase1:
                            continue
                        for ci in range(CI_CHUNKS):
                            mb = ps_mb.tile([128, TT], f32, tag="mb")
                            nc.tensor.matmul(mb[:], g2_row[0:1, ts(ci, 128)],
                                             r_row[0:1, ts(j, TT)],
                                             start=True, stop=True)
                            u = scr.tile([128, TT], f32, tag="scrb")
                            idx = j * CI_CHUNKS + ci
                            nc.vector.tensor_tensor(
                                u[:], x_sb[:, ci, ts(j, TT)], mb[:],
                                op=Alu.mult)
                            nc.vector.tensor_reduce(
                                coll[:, idx:idx + 1], u[:],
                                axis=mybir.AxisListType.X, op=Alu.max,
                                apply_absolute_value=True)

                # ---- weight pass A: sum(|w|) for mean (overlaps 1b) ----
                wsacc = None
                if skip_w:
                    wsacc = scp.tile([128, 1], f32, tag="sc")
                    nc.vector.memset(wsacc[:], 1.0)
                for e in range(0 if skip_w else NW):
                    wt_e = wstga.tile([128, WCH], f32, tag="wstga")
                    nc.scalar.dma_start(wt_e[:], wt_t[:, ts(e, WCH)])
                    wsq = scp.tile([128, 1], f32, tag="sc")
                    nc.scalar.activation(wt_e[:], wt_e[:], Act.Abs,
                                         accum_out=wsq[:])
                    if wsacc is None:
                        wsacc = wsq
                    else:
                        nxt = scp.tile([128, 1], f32, tag="sc")
                        nc.vector.tensor_tensor(nxt[:], wsacc[:], wsq[:],
                                                op=Alu.add)
                        wsacc = nxt
                wsum_ps = ps_small.tile([1, 1], f32, tag="ssq")
                nc.tensor.matmul(wsum_ps[:], wsacc[:], ones_col[:, 0:1],
                                 start=True, stop=True)
                wscale = scp.tile([1, 1], f32, tag="sc")
                nc.scalar.copy(wscale[:], wsum_ps[:])
                nc.vector.tensor_scalar(wscale[:], wscale[:], 1.0 / W_COUNT,
                                        EPS_SCALE, op0=Alu.mult, op1=Alu.max)
                winv = scp.tile([1, 1], f32, tag="sc")
                nc.vector.reciprocal(winv[:], wscale[:])
                winv_col = scp.tile([128, 1], f32, tag="sc")
                nc.gpsimd.partition_broadcast(winv_col[:], winv[:])

                prev = amaxp.tile([128, 1], f32, tag="amax")
                nc.vector.tensor_reduce(prev[:], coll[:],
                                        axis=mybir.AxisListType.X, op=Alu.max)

            # x_sb freed here.
            amax_all = scp.tile([128, 1], f32, tag="sc")
            nc.gpsimd.partition_all_reduce(amax_all[:], prev[:], channels=128,
                                           reduce_op=bass_isa.ReduceOp.max)
            nc.sync.dma_start(cc_in[:], amax_all[:])
            if n_cores > 1:
                nc.gpsimd.collective_compute(
                    "AllReduce", Alu.max,
                    replica_groups=[list(range(n_cores))],
                    ins=[cc_in[:].opt()], outs=[cc_out[:].opt()])
            else:
                nc.sync.dma_start(cc_out[:], cc_in[:])

            v_raw = scp.tile([1, 1], f32, tag="sc")
            nc.sync.dma_start(v_raw[0:1, 0:1],
                              cc_out[0:1].rearrange("(a d) -> a d", a=1))
            qscale = scp.tile([1, 1], f32, tag="sc")
            nc.vector.tensor_scalar_max(qscale[:], v_raw[:], EPS_SCALE)
            qinv = scp.tile([1, 1], f32, tag="sc")
            nc.vector.reciprocal(qinv[:], qscale[:])
            q254 = scp.tile([1, 1], f32, tag="sc")
            nc.vector.tensor_scalar_mul(q254[:], qinv[:], 2.0 * QP)
            g2q_row = singles.tile([1, C], f32)
            nc.vector.tensor_scalar_mul(g2q_row[:], g_row[:], q254[:])
            # final output scale = wscale * qscale / 127
            fs = scp.tile([1, 1], f32, tag="sc")
            nc.vector.tensor_tensor(fs[:], wscale[:], qscale[:], op=Alu.mult)
            nc.vector.tensor_scalar_mul(fs[:], fs[:], 1.0 / QP)
            fs_col = scp.tile([128, 1], f32, tag="sc")
            nc.gpsimd.partition_broadcast(fs_col[:], fs[:])
            # ---------------- phase 2 pools (open after x_sb freed) ---------
            wstgb = stk.enter_context(tc.tile_pool(name="wstgb", bufs=2))
            wqp = stk.enter_context(tc.tile_pool(name="wqp", bufs=1))
            qp = stk.enter_context(tc.tile_pool(name="qp", bufs=1))
            xsp = stk.enter_context(tc.tile_pool(name="xsp", bufs=6))
            outp = stk.enter_context(tc.tile_pool(name="outp", bufs=4))

            # ---------------- phase 2: quantize activations -----------------
            q_sb = qp.tile([128, CI_CHUNKS, t_len], bf16)
            for j in range(0 if skip_quant else NT):
                for ci in range(CI_CHUNKS):
                    xs = xsp.tile([128, TT], f32, tag="xs")
                    nc.sync.dma_start(xs[:], xv[:, ci, ts(j, TT)])
                    mb2 = ps_conv.tile([128, TT], f32, tag="conv")
                    nc.tensor.matmul(mb2[:], g2q_row[0:1, ts(ci, 128)],
                                     r_row[0:1, ts(j, TT)],
                                     start=True, stop=True)
                    u2 = scr.tile([128, TT], f32, tag="scr")
                    nc.vector.tensor_tensor(u2[:], xs[:], mb2[:], op=Alu.mult)
                    last_quant = nc.vector.tensor_scalar(
                        q_sb[:, ci, ts(j, TT)], u2[:], C_MAGIC, C_MAGIC,
                        op0=Alu.add, op1=Alu.subtract)

            # ---------------- weight pass B: ternary quant ------------------
            wq_sb = wqp.tile([128, WQ_F], bf16)
            for e in range(0 if skip_w else NW):
                w8 = wstgb.tile([128, WCH], f32, tag="wstgb")
                nc.scalar.dma_start(w8[:], wt_t[:, ts(e, WCH)])
                nc.vector.tensor_scalar(w8[:], w8[:], winv_col[:], 1.0,
                                        op0=Alu.mult, op1=Alu.min)
                nc.vector.tensor_scalar(w8[:], w8[:], -1.0, C_MAGIC,
                                        op0=Alu.max, op1=Alu.add)
                nc.vector.tensor_scalar(wq_sb[:, ts(e, WCH)], w8[:],
                                        C_MAGIC, None, op0=Alu.subtract)
            wqv = wq_sb[:].rearrange("p (cb k ci o) -> p cb k ci o",
                                     cb=CB_BLOCKS, k=K, ci=CI_CHUNKS)

            # ---------------- conv: 28 shifted matmuls per tile -------------
            # Tap order puts k=3 (always full width) first so the start=True
            # matmul covers the whole PSUM tile.
            tap_order = [3, 0, 1, 2, 4, 5, 6]
            from concourse.bass import _add_dep_helper
            for cb in range(CB_BLOCKS if not skip_conv else 0):
                for j in range(NT):
                    cps = ps_conv.tile([128, TT], f32, tag="conv")
                    n_mm = 0
                    for k in tap_order:
                        lo_data = j * TT + k - PAD
                        out_lo = max(0, -lo_data)
                        out_hi = TT - max(0, lo_data + TT - t_len)
                        for ci in range(CI_CHUNKS):
                            mm = nc.tensor.matmul(
                                cps[:, out_lo:out_hi],
                                wqv[:, cb, k, ci, :],
                                q_sb[:, ci,
                                     lo_data + out_lo:lo_data + out_hi],
                                start=(n_mm == 0),
                                stop=(n_mm == K * CI_CHUNKS - 1))
                            if n_mm == 0 and not skip_quant:
                                # keep the conv MM stream dense: start only
                                # after quantization fully completes
                                _add_dep_helper(mm.ins, last_quant.ins, True,
                                                "conv after quant")
                            n_mm += 1
                    osb = outp.tile([128, TT], f32)
                    nc.scalar.activation(osb[:], cps[:], Act.Copy,
                                         scale=fs_col[:])
                    nc.scalar.dma_start(out_t[ts(cb, 128), ts(j, TT)], osb[:])

    nc.compile()
    return nc


def _prep_weight(weight: np.ndarray) -> np.ndarray:
    # WT[p, cb, k, ci, o'] = weight[cb*128+o', ci*128+p, k], flattened to
    # (128, 14336) so lhsT tiles are contiguous slices.
    w = np.ascontiguousarray(weight.astype(np.float32, copy=False))
    w5 = w.reshape(CB_BLOCKS, 128, CI_CHUNKS, 128, K)  # [cb, o', ci, p, k]
    wt = w5.transpose(3, 0, 4, 2, 1)  # [p, cb, k, ci, o']
    return np.ascontiguousarray(wt.reshape(128, -1))


def kernel(x: np.ndarray, weight: np.ndarray, gamma: np.ndarray) -> np.ndarray:
    from concourse.bass_utils import run_bass_kernel_spmd

    key = ("full", N_CORES, T)
    if key not in _CACHE:
        _CACHE[key] = _build(N_CORES, T)
    nc = _CACHE[key]

    wt = _prep_weight(weight)
    g = np.ascontiguousarray(gamma.astype(np.float32, copy=False))
    in_maps = [
        {"x": np.ascontiguousarray(x[b].astype(np.float32, copy=False)),
         "wt": wt, "g": g}
        for b in range(N_CORES)
    ]
    res = run_bass_kernel_spmd(nc, in_maps, list(range(N_CORES)))
    out = np.stack([res.results[b]["out"] for b in range(N_CORES)], axis=0)
    return out



# revision 7
# speedup vs baseline: 1.6471x; 1.6471x over previous
"""BitConv1d Trainium2 kernel (fp8 DoubleRow version).

Computes, for x:(8,512,8192) f32, weight:(512,512,7) f32, gamma:(512,) f32:
  rms  = sqrt(mean(x^2, channel) + 1e-6)          (per b,t)
  xn   = x / rms * gamma
  s    = max(|xn|) over the FULL batch  (clamped to >= 1e-5)
  q    = round(clip(xn/s*127, -128, 127))         (8-bit act quant)
  ws   = max(mean(|w|), 1e-5); wq = round(clip(w/ws, -1, 1))  (ternary)
  out  = conv1d(q * s/127, wq, pad 3) * ws

Strategy: data-parallel over batch across 8 NeuronCores (1 batch element
per core), weights replicated; AllReduce(max) for the global activation
scale. The conv runs as fp8e4 DoubleRow matmuls with an exact integer
split q = qh + ql (qh = round8(q), a multiple of 8 with |qh| <= 128;
ql = q - qh in [-4,4]); these values and the ternary weights are exactly
representable in fp8e4, and f32 PSUM accumulation of integers < 2^24 is
exact, so the conv equals the integer conv, scaled afterward. Rounding
uses (x + 1.5*2^23) - 1.5*2^23 (round-half-even) and the
round-to-multiple-of-8 variant with 1.5*2^26.
"""

import sys

sys.path.insert(0, "/opt/trn_rl_repo")

import numpy as np

N_CORES = 8
B, C, T = 8, 512, 8192
CO, K = 512, 7
CI = 4          # in-channel chunks of 128
CB = 4          # out-channel blocks of 128
TT = 512        # conv time-tile (psum bank width)
PAD = 3
GRP = 4         # j-tiles per phase-1 group
GT = GRP * TT   # 2048 cols per group
QG = 512        # phase-2 quant grain

EPS_NORM = 1e-6
EPS_SCALE = 1e-5
QP = 127.0
C1 = 12582912.0          # 1.5*2^23: round to int
D8 = 7.5 * 2 ** 23       # C8-C1 where C8 = 1.5*2^26: round to multiple of 8
W_COUNT = CO * C * K
WQ_F = CB * K * CI * 128  # 14336 weight columns
NWA = 16                  # pass-A chunks
NWB = 8                   # pass-B chunks

_CACHE = {}


def _build(n_cores: int, t_len: int):
    import contextlib

    import concourse.bacc as bacc
    import concourse.bass as bass
    import concourse.tile as tile
    from concourse import bass_isa, mybir

    f32 = mybir.dt.float32
    fp8 = mybir.dt.float8e4
    Alu = mybir.AluOpType
    Act = mybir.ActivationFunctionType
    DR = mybir.MatmulPerfMode.DoubleRow
    ts = bass.ts

    nt = t_len // TT
    ng = nt // GRP
    nq = t_len // QG
    WA = WQ_F // NWA  # 896
    WB = WQ_F // NWB  # 1792

    nc = bacc.Bacc("TRN2", target_bir_lowering=False, debug=False,
                   num_devices=n_cores)

    x_t = nc.dram_tensor("x", [C, t_len], f32, kind="ExternalInput")
    wt_t = nc.dram_tensor("wt", [128, WQ_F], f32, kind="ExternalInput")
    g_t = nc.dram_tensor("g", [C], f32, kind="ExternalInput")
    out_t = nc.dram_tensor("out", [CO, t_len], f32, kind="ExternalOutput")

    xv = x_t[:].rearrange("(c p) t -> p c t", p=128)  # chunk-major channels

    with tile.TileContext(nc) as tc:
        with contextlib.ExitStack() as stk:
            singles = stk.enter_context(tc.tile_pool(name="singles", bufs=1))
            sc = stk.enter_context(tc.tile_pool(name="sc", bufs=24))
            rlinp = stk.enter_context(tc.tile_pool(name="rlin", bufs=1))
            wqp = stk.enter_context(tc.tile_pool(name="wqp", bufs=1))
            wsta = stk.enter_context(tc.tile_pool(name="wsta", bufs=2))
            rrowp = stk.enter_context(tc.tile_pool(name="rrow", bufs=2))
            dramp = stk.enter_context(
                tc.tile_pool(name="dram", bufs=1, space="DRAM"))
            ps_ssq = stk.enter_context(
                tc.tile_pool(name="ps_ssq", bufs=2, space="PSUM"))
            ps_w = stk.enter_context(
                tc.tile_pool(name="ps_w", bufs=1, space="PSUM"))
            ps_conv = stk.enter_context(
                tc.tile_pool(name="ps_conv", bufs=4, space="PSUM"))

            ones_col = singles.tile([128, 1], f32)
            nc.vector.memset(ones_col[:], 1.0)
            eps_col = singles.tile([128, 1], f32)
            nc.vector.memset(eps_col[:], EPS_NORM)
            c1_col = singles.tile([128, 1], f32)
            nc.vector.memset(c1_col[:], C1)
            mc1_col = singles.tile([128, 1], f32)
            nc.vector.memset(mc1_col[:], -C1)
            zmov = singles.tile([128, GRP * 4], f32)
            nc.vector.memset(zmov[:], 0.0)
            g_cm = singles.tile([128, CI], f32)  # gamma, chunk-major
            nc.sync.dma_start(g_cm[:], g_t[:].rearrange("(c p) -> p c", p=128))
            gabs_cm = singles.tile([128, CI], f32)  # 2*|gamma|
            nc.scalar.activation(gabs_cm[:], g_cm[:], Act.Abs, scale=2.0)

            r_dram = dramp.tile([t_len], f32)
            cc_in = dramp.tile([128], f32)
            cc_out = dramp.tile([128], f32)
            rd_pb = r_dram[:].rearrange("(b p) -> p b", p=128)
            rd_row = r_dram[:].rearrange("(a d) -> a d", a=1)

            # rcol layout: rcol[p, b] = value for t = b*128 + p
            NB = t_len // 128  # 64 t-blocks
            rcol = singles.tile([128, NB], f32)
            mcol = singles.tile([128, NB], f32)
            s0col = singles.tile([128, NB], f32)
            tdcol = singles.tile([128, NB], f32)
            rhcol = singles.tile([128, NB], f32)
            # R = rhalf broadcast to all partitions; persists into phase 2
            R = rlinp.tile([128, t_len], f32)
            coll = singles.tile([128, CI, ng], f32)
            wq8 = wqp.tile([128, WQ_F], fp8)

            # ---- weight pass A: sum|w| (Act engine, overlaps phase 1) ----
            wsacc = None
            for e in range(NWA):
                wt_e = wsta.tile([128, WA], f32, tag="wsta")
                nc.gpsimd.dma_start(wt_e[:], wt_t[:, ts(e, WA)])
                wsq = sc.tile([128, 1], f32, tag="sc")
                nc.scalar.activation(wt_e[:], wt_e[:], Act.Abs,
                                     accum_out=wsq[:])
                if wsacc is None:
                    wsacc = wsq
                else:
                    nxt = sc.tile([128, 1], f32, tag="sc")
                    nc.vector.tensor_tensor(nxt[:], wsacc[:], wsq[:],
                                            op=Alu.add)
                    wsacc = nxt
            wsum_ps = ps_w.tile([1, 1], f32, tag="wsum")
            nc.tensor.matmul(wsum_ps[:], wsacc[:], ones_col[:, 0:1],
                             start=True, stop=True)
            wscale = sc.tile([1, 1], f32, tag="sc")
            nc.scalar.copy(wscale[:], wsum_ps[:])
            nc.vector.tensor_scalar(wscale[:], wscale[:], 1.0 / W_COUNT,
                                    EPS_SCALE, op0=Alu.mult, op1=Alu.max)
            winv = sc.tile([1, 1], f32, tag="sc")
            nc.vector.reciprocal(winv[:], wscale[:])
            winv_col = sc.tile([128, 1], f32, tag="sc")
            nc.gpsimd.partition_broadcast(winv_col[:], winv[:])

            # ---------------- phase 1: rms + local max ----------------
            with tc.tile_pool(name="xina", bufs=2) as xinp, \
                 tc.tile_pool(name="x2a", bufs=2) as x2p, \
                 tc.tile_pool(name="vs", bufs=3) as vsp:
                for G in range(ng):
                    xg = xinp.tile([128, CI, GT], f32, tag="xin")
                    sp = ps_ssq.tile([128, GRP * 4], f32, tag="ssq")
                    for j in range(GRP):
                        jj = G * GRP + j
                        nc.sync.dma_start(xg[:, :, ts(j, TT)],
                                          xv[:, :, ts(jj, TT)])
                        x2 = x2p.tile([128, CI, TT], f32, tag="x2")
                        nc.scalar.activation(x2[:], xg[:, :, ts(j, TT)],
                                             Act.Square)
                        if j == 0:
                            # zero the whole psum bank once (start=True),
                            # then accumulate columns with start=False.
                            nc.tensor.matmul(sp[:], x2[:, 0, 0:128], zmov[:],
                                             start=True, stop=False)
                        # ssq over channels: tall-stationary colsums
                        for b in range(4):  # 4 t-blocks per j-tile
                            col = j * 4 + b
                            for ci in range(CI):
                                nc.tensor.matmul(
                                    sp[:, col:col + 1],
                                    x2[:, ci, ts(b, 128)], ones_col[:],
                                    start=False,
                                    stop=(j == GRP - 1 and b == 3
                                          and ci == CI - 1))
                    gs = slice(G * GRP * 4, (G + 1) * GRP * 4)
                    nc.vector.tensor_scalar_mul(rcol[:, gs], sp[:], 1.0)
                    # rms math for this group's 16 blocks (Newton refined)
                    nc.vector.tensor_scalar(mcol[:, gs], rcol[:, gs], 1.0 / C,
                                            EPS_NORM, op0=Alu.mult,
                                            op1=Alu.add)
                    nc.scalar.activation(s0col[:, gs], rcol[:, gs], Act.Sqrt,
                                         bias=eps_col[:], scale=1.0 / C)
                    nc.vector.reciprocal(tdcol[:, gs], s0col[:, gs])
                    nc.vector.tensor_tensor(tdcol[:, gs], mcol[:, gs],
                                            tdcol[:, gs], op=Alu.mult)
                    nc.vector.tensor_tensor(tdcol[:, gs], tdcol[:, gs],
                                            s0col[:, gs], op=Alu.add)
                    nc.vector.reciprocal(rhcol[:, gs], tdcol[:, gs])
                    # bounce rhalf to DRAM, read back linear, broadcast
                    nc.gpsimd.dma_start(rd_pb[:, gs], rhcol[:, gs])
                    r_row = rrowp.tile([1, GT], f32, tag="rrow")
                    nc.gpsimd.dma_start(r_row[:], rd_row[0:1, ts(G, GT)])
                    nc.gpsimd.partition_broadcast(R[:, ts(G, GT)], r_row[:])
                    # local max |x*rhalf| per (ci, G); |gamma| folded later
                    for ci in range(CI):
                        v = vsp.tile([128, GT], f32, tag="v")
                        eng = nc.vector if ci < 2 else nc.gpsimd
                        eng.tensor_tensor(v[:], xg[:, ci, :],
                                          R[:, ts(G, GT)], op=Alu.mult)
                        nc.vector.tensor_reduce(coll[:, ci, G:G + 1], v[:],
                                                axis=mybir.AxisListType.X,
                                                op=Alu.max,
                                                apply_absolute_value=True)

            # ---- global max + AllReduce ----
            gcoll = sc.tile([128, CI, ng], f32, tag="gcoll")
            for ci in range(CI):
                nc.vector.tensor_scalar(gcoll[:, ci, :], coll[:, ci, :],
                                        gabs_cm[:, ci:ci + 1], None,
                                        op0=Alu.mult)
            mx = sc.tile([128, 1], f32, tag="sc")
            nc.vector.tensor_reduce(mx[:], gcoll[:].rearrange("p a b -> p (a b)"),
                                    axis=mybir.AxisListType.X, op=Alu.max)
            amax_all = sc.tile([128, 1], f32, tag="sc")
            nc.gpsimd.partition_all_reduce(amax_all[:], mx[:], channels=128,
                                           reduce_op=bass_isa.ReduceOp.max)
            nc.sync.dma_start(cc_in[:], amax_all[:])
            if n_cores > 1:
                nc.gpsimd.collective_compute(
                    "AllReduce", Alu.max,
                    replica_groups=[list(range(n_cores))],
                    ins=[cc_in[:].opt()], outs=[cc_out[:].opt()])
            else:
                nc.sync.dma_start(cc_out[:], cc_in[:])

            # ---- weight pass B: ternary quant to fp8 (during AllReduce) ----
            for e in range(NWB):
                w8 = wsta.tile([128, WB], f32, tag="wstb")
                nc.gpsimd.dma_start(w8[:], wt_t[:, ts(e, WB)])
                nc.vector.tensor_scalar(w8[:], w8[:], winv_col[:], C1,
                                        op0=Alu.mult, op1=Alu.add)
                nc.vector.tensor_scalar(w8[:], w8[:], C1 + 1.0, C1 - 1.0,
                                        op0=Alu.min, op1=Alu.max)
                nc.vector.tensor_scalar(wq8[:, ts(e, WB)], w8[:], C1, None,
                                        op0=Alu.subtract)
            wqv = wq8[:].rearrange("p (cb k ci o) -> p cb k ci o",
                                   cb=CB, k=K, ci=CI)

            # ---- post-AllReduce scalar math ----
            v_raw = sc.tile([1, 1], f32, tag="sc")
            nc.sync.dma_start(v_raw[0:1, 0:1],
                              cc_out[0:1].rearrange("(a d) -> a d", a=1))
            qscale = sc.tile([1, 1], f32, tag="sc")
            nc.vector.tensor_scalar_max(qscale[:], v_raw[:], EPS_SCALE)
            qinv = sc.tile([1, 1], f32, tag="sc")
            nc.vector.reciprocal(qinv[:], qscale[:])
            q254 = sc.tile([1, 1], f32, tag="sc")
            nc.vector.tensor_scalar_mul(q254[:], qinv[:], 2.0 * QP)
            q254_col = sc.tile([128, 1], f32, tag="sc")
            nc.gpsimd.partition_broadcast(q254_col[:], q254[:])
            gq_cm = sc.tile([128, CI], f32, tag="gq")
            nc.vector.tensor_scalar(gq_cm[:], g_cm[:], q254_col[:], None,
                                    op0=Alu.mult)
            fs = sc.tile([1, 1], f32, tag="sc")
            nc.vector.tensor_tensor(fs[:], wscale[:], qscale[:], op=Alu.mult)
            nc.vector.tensor_scalar_mul(fs[:], fs[:], 1.0 / QP)
            fs_col = sc.tile([128, 1], f32, tag="sc")
            nc.gpsimd.partition_broadcast(fs_col[:], fs[:])

            # ---------------- phase 2: quantize + conv ----------------
            qhp = stk.enter_context(tc.tile_pool(name="qhp", bufs=1))
            qlp = stk.enter_context(tc.tile_pool(name="qlp", bufs=1))
            xinb = stk.enter_context(tc.tile_pool(name="xinb", bufs=3))
            tscr = stk.enter_context(tc.tile_pool(name="tscr", bufs=3))
            outp = stk.enter_context(tc.tile_pool(name="outp", bufs=4))

            qh_sb = qhp.tile([128, CI, t_len], fp8)
            ql_sb = qlp.tile([128, CI, t_len], fp8)

            def quant_grain(gq):
                xg2 = xinb.tile([128, CI, QG], f32, tag="xin2")
                nc.sync.dma_start(xg2[:], xv[:, :, ts(gq, QG)])
                for ci in range(CI):
                    u2 = tscr.tile([128, QG], f32, tag="u2")
                    nc.vector.tensor_tensor(u2[:], xg2[:, ci, :],
                                            R[:, ts(gq, QG)], op=Alu.mult)
                    t1 = tscr.tile([128, QG], f32, tag="t1")
                    nc.scalar.activation(t1[:], u2[:], Act.Identity,
                                         bias=c1_col[:],
                                         scale=gq_cm[:, ci:ci + 1])
                    t3 = tscr.tile([128, QG], f32, tag="t3")
                    nc.vector.tensor_scalar(t3[:], t1[:], D8, D8,
                                            op0=Alu.add, op1=Alu.subtract)
                    nc.scalar.activation(qh_sb[:, ci, ts(gq, QG)], t3[:],
                                         Act.Identity, bias=mc1_col[:],
                                         scale=1.0)
                    nc.vector.tensor_tensor(ql_sb[:, ci, ts(gq, QG)], t1[:],
                                            t3[:], op=Alu.subtract)

            def conv_tile(cb, j):
                cps = ps_conv.tile([128, TT], f32, tag="conv")
                n_mm = 0
                for k in (3, 0, 1, 2, 4, 5, 6):
                    lo_data = j * TT + k - PAD
                    out_lo = max(0, -lo_data)
                    out_hi = TT - max(0, lo_data + TT - t_len)
                    a, b = lo_data + out_lo, lo_data + out_hi
                    for src in (qh_sb, ql_sb):
                        for cp in range(2):
                            nc.tensor.matmul(
                                cps[:, out_lo:out_hi],
                                wqv[:, cb, k, 2 * cp:2 * cp + 2, :],
                                src[:, 2 * cp:2 * cp + 2, a:b],
                                perf_mode=DR,
                                start=(n_mm == 0), stop=(n_mm == 27))
                            n_mm += 1
                osb = outp.tile([128, TT], f32)
                nc.scalar.activation(osb[:], cps[:], Act.Copy,
                                     scale=fs_col[:])
                nc.sync.dma_start(out_t[ts(cb, 128), ts(j, TT)], osb[:])

            # software pipeline: quant stays one group ahead of conv
            QPG = GT // QG  # quant grains per group
            for G in range(ng):
                for gq in range(G * QPG, (G + 1) * QPG):
                    quant_grain(gq)
                if G >= 1:
                    for cb in range(CB):
                        for j in range((G - 1) * GRP, G * GRP):
                            conv_tile(cb, j)
            for cb in range(CB):
                for j in range((ng - 1) * GRP, nt):
                    conv_tile(cb, j)

    nc.compile()
    return nc


def _prep_weight(weight: np.ndarray) -> np.ndarray:
    # WT[p, cb, k, ci, o'] = weight[cb*128+o', ci*128+p, k], flattened to
    # (128, 14336) so lhsT tiles are contiguous slices.
    w = np.ascontiguousarray(weight.astype(np.float32, copy=False))
    w5 = w.reshape(CB, 128, CI, 128, K)  # [cb, o', ci, p, k]
    wt = w5.transpose(3, 0, 4, 2, 1)  # [p, cb, k, ci, o']
    return np.ascontiguousarray(wt.reshape(128, -1))


def kernel(x: np.ndarray, weight: np.ndarray, gamma: np.ndarray) -> np.ndarray:
    from concourse.bass_utils import run_bass_kernel_spmd

    key = ("full", N_CORES, T)
    if key not in _CACHE:
        _CACHE[key] = _build(N_CORES, T)
    nc = _CACHE[key]

    wt = _prep_weight(weight)
    g = np.ascontiguousarray(gamma.astype(np.float32, copy=False))
    in_maps = [
        {"x": np.ascontiguousarray(x[b].astype(np.float32, copy=False)),
         "wt": wt, "g": g}
        for b in range(N_CORES)
    ]
    res = run_bass_kernel_spmd(nc, in_maps, list(range(N_CORES)))
    out = np.stack([res.results[b]["out"] for b in range(N_CORES)], axis=0)
    return out


# revision 26
# speedup vs baseline: 1.7786x; 1.0798x over previous
"""BitConv1d Trainium2 kernel (fp8 DoubleRow version).

Computes, for x:(8,512,8192) f32, weight:(512,512,7) f32, gamma:(512,) f32:
  rms  = sqrt(mean(x^2, channel) + 1e-6)          (per b,t)
  xn   = x / rms * gamma
  s    = max(|xn|) over the FULL batch  (clamped to >= 1e-5)
  q    = round(clip(xn/s*127, -128, 127))         (8-bit act quant)
  ws   = max(mean(|w|), 1e-5); wq = round(clip(w/ws, -1, 1))  (ternary)
  out  = conv1d(q * s/127, wq, pad 3) * ws

Strategy: data-parallel over batch across 8 NeuronCores (1 batch element
per core), weights replicated; AllReduce(max) for the global activation
scale. The conv runs as fp8e4 DoubleRow matmuls with an exact integer
split q = qh + ql (qh = round8(q), a multiple of 8 with |qh| <= 128;
ql = q - qh in [-4,4]); these values and the ternary weights are exactly
representable in fp8e4, and f32 PSUM accumulation of integers < 2^24 is
exact, so the conv equals the integer conv, scaled afterward. Rounding
uses (x + 1.5*2^23) - 1.5*2^23 (round-half-even) and the
round-to-multiple-of-8 variant with 1.5*2^26.
"""

import sys

sys.path.insert(0, "/opt/trn_rl_repo")

import numpy as np

N_CORES = 8
B, C, T = 8, 512, 8192
CO, K = 512, 7
CI = 4          # in-channel chunks of 128
CB = 4          # out-channel blocks of 128
TT = 512        # conv time-tile (psum bank width)
PAD = 3
GRP = 4         # j-tiles per phase-1 group
GT = GRP * TT   # 2048 cols per group
QG = 512        # phase-2 quant grain

EPS_NORM = 1e-6
EPS_SCALE = 1e-5
QP = 127.0
C1 = 12582912.0          # 1.5*2^23: round to int
D8 = 7.5 * 2 ** 23       # C8-C1 where C8 = 1.5*2^26: round to multiple of 8
W_COUNT = CO * C * K
WQ_F = CB * K * CI * 128  # 14336 weight columns
NWA = 16                  # pass-A chunks
NWB = 8                   # pass-B chunks

_CACHE = {}


def _build(n_cores: int, t_len: int):
    import contextlib

    import concourse.bacc as bacc
    import concourse.bass as bass
    import concourse.tile as tile
    from concourse import bass_isa, mybir

    f32 = mybir.dt.float32
    fp8 = mybir.dt.float8e4
    Alu = mybir.AluOpType
    Act = mybir.ActivationFunctionType
    DR = mybir.MatmulPerfMode.DoubleRow
    ts = bass.ts

    nt = t_len // TT
    ng = nt // GRP
    nq = t_len // QG
    WA = WQ_F // NWA  # 896
    WB = WQ_F // NWB  # 1792

    nc = bacc.Bacc("TRN2", target_bir_lowering=False, debug=False,
                   num_devices=n_cores)

    x_t = nc.dram_tensor("x", [C, t_len], f32, kind="ExternalInput")
    wt_t = nc.dram_tensor("wt", [128, WQ_F], f32, kind="ExternalInput")
    g_t = nc.dram_tensor("g", [C], f32, kind="ExternalInput")
    out_t = nc.dram_tensor("out", [CO, t_len], f32, kind="ExternalOutput")

    xv = x_t[:].rearrange("(c p) t -> p c t", p=128)  # chunk-major channels

    with tile.TileContext(nc) as tc:
        with contextlib.ExitStack() as stk:
            singles = stk.enter_context(tc.tile_pool(name="singles", bufs=1))
            sc = stk.enter_context(tc.tile_pool(name="sc", bufs=24))
            rlinp = stk.enter_context(tc.tile_pool(name="rlin", bufs=1))
            wqp = stk.enter_context(tc.tile_pool(name="wqp", bufs=1))
            wsta = stk.enter_context(tc.tile_pool(name="wsta", bufs=2))
            dramp = stk.enter_context(
                tc.tile_pool(name="dram", bufs=1, space="DRAM"))
            drbp = stk.enter_context(
                tc.tile_pool(name="drb", bufs=4, space="DRAM"))
            ps_ssq = stk.enter_context(
                tc.tile_pool(name="ps_ssq", bufs=2, space="PSUM"))
            ps_w = stk.enter_context(
                tc.tile_pool(name="ps_w", bufs=1, space="PSUM"))
            ps_conv = stk.enter_context(
                tc.tile_pool(name="ps_conv", bufs=4, space="PSUM"))

            ones_col = singles.tile([128, 1], f32)
            nc.vector.memset(ones_col[:], 1.0)
            eps_col = singles.tile([128, 1], f32)
            nc.vector.memset(eps_col[:], EPS_NORM)
            c1_col = singles.tile([128, 1], f32)
            nc.vector.memset(c1_col[:], C1)
            mc1_col = singles.tile([128, 1], f32)
            nc.vector.memset(mc1_col[:], -C1)
            zmov = singles.tile([128, GRP * 4], f32)
            nc.vector.memset(zmov[:], 0.0)
            g_cm = singles.tile([128, CI], f32)  # gamma, chunk-major
            nc.sync.dma_start(g_cm[:], g_t[:].rearrange("(c p) -> p c", p=128))
            gabs_cm = singles.tile([128, CI], f32)  # 2*|gamma|
            nc.scalar.activation(gabs_cm[:], g_cm[:], Act.Abs, scale=2.0)

            cc_in = dramp.tile([128], f32)
            cc_out = dramp.tile([128], f32)

            # R = rhalf broadcast to all partitions; persists into phase 2
            R = rlinp.tile([128, t_len], f32)
            coll = singles.tile([128, CI, nt], f32)

            # ---------------- phase 1: rms + local max ----------------
            # j-tile granular pipeline: per 512-col tile, the serial chain
            # dma -> square -> ssq colsums -> rms -> bounce -> broadcast ->
            # v=x*R -> reduce has ~6us latency, hidden by 6 x-tile buffers.
            with tc.tile_pool(name="xina", bufs=6) as xinp, \
                 tc.tile_pool(name="x2a", bufs=3) as x2p, \
                 tc.tile_pool(name="rms", bufs=4) as rmsp, \
                 tc.tile_pool(name="vs", bufs=4) as vsp:
                vq = []

                def emit_vred(jj, xg):
                    for ci in range(CI):
                        v = vsp.tile([128, TT], f32, tag="v")
                        eng = nc.vector if ci < 2 else nc.gpsimd
                        eng.tensor_tensor(v[:], xg[:, ci, :],
                                          R[:, ts(jj, TT)], op=Alu.mult)
                        nc.vector.tensor_reduce(coll[:, ci, jj:jj + 1], v[:],
                                                axis=mybir.AxisListType.X,
                                                op=Alu.max,
                                                apply_absolute_value=True)

                for jj in range(nt):
                    xg = xinp.tile([128, CI, TT], f32, tag="xin")
                    nc.sync.dma_start(xg[:], xv[:, :, ts(jj, TT)])
                    x2 = x2p.tile([128, CI, TT], f32, tag="x2")
                    nc.scalar.activation(x2[:], xg[:], Act.Square)
                    sp = ps_ssq.tile([128, 4], f32, tag="ssq")
                    nc.tensor.matmul(sp[:], x2[:, 0, 0:128], zmov[:, 0:4],
                                     start=True, stop=False)
                    for b in range(4):  # 4 t-blocks per j-tile
                        for ci in range(CI):
                            nc.tensor.matmul(
                                sp[:, b:b + 1],
                                x2[:, ci, ts(b, 128)], ones_col[:],
                                start=False,
                                stop=(b == 3 and ci == CI - 1))
                    rc = rmsp.tile([128, 4], f32, tag="rc")
                    nc.scalar.copy(rc[:], sp[:])
                    # rms math for this tile's 4 blocks (Newton refined)
                    mc = rmsp.tile([128, 4], f32, tag="mc")
                    nc.vector.tensor_scalar(mc[:], rc[:], 1.0 / C,
                                            EPS_NORM, op0=Alu.mult,
                                            op1=Alu.add)
                    s0 = rmsp.tile([128, 4], f32, tag="s0")
                    nc.scalar.activation(s0[:], rc[:], Act.Sqrt,
                                         bias=eps_col[:], scale=1.0 / C)
                    td = rmsp.tile([128, 4], f32, tag="td")
                    nc.vector.reciprocal(td[:], s0[:])
                    nc.vector.tensor_tensor(td[:], mc[:], td[:], op=Alu.mult)
                    nc.vector.tensor_tensor(td[:], td[:], s0[:], op=Alu.add)
                    rh = rmsp.tile([128, 4], f32, tag="rh")
                    nc.vector.reciprocal(rh[:], td[:])
                    # bounce rhalf to DRAM, broadcast back to all rows
                    rb = drbp.tile([TT], f32, tag="rb")
                    nc.scalar.dma_start(
                        rb[:].rearrange("(b p) -> p b", p=128), rh[:])
                    nc.gpsimd.dma_start(
                        R[:, ts(jj, TT)],
                        rb[:].rearrange("(a d) -> a d", a=1)
                        .partition_broadcast(128))
                    # local max |x*rhalf| per (ci, tile), emitted two
                    # tiles late so the R broadcast has already landed and
                    # DVE's in-order stream never waits on it
                    vq.append((jj, xg))
                    if len(vq) > 2:
                        emit_vred(*vq.pop(0))
                for jj, xg in vq:
                    emit_vred(jj, xg)

            # ---- weight pass A: DMAs early (transfers overlap collective);
            # each chunk's |w| column-sum lands in its own wsqs column ----
            wsqs = sc.tile([128, NWA], f32, tag="wsqs")
            for e in range(NWA):
                wt_e = wsta.tile([128, WB], f32, tag="wstb")
                nc.sync.dma_start(wt_e[:, 0:WA], wt_t[:, ts(e, WA)])
                nc.scalar.activation(wt_e[:, 0:WA], wt_e[:, 0:WA], Act.Abs,
                                     accum_out=wsqs[:, e:e + 1])

            # ---- global max + AllReduce ----
            gcoll = sc.tile([128, CI, nt], f32, tag="gcoll")
            for ci in range(CI):
                nc.vector.tensor_scalar(gcoll[:, ci, :], coll[:, ci, :],
                                        gabs_cm[:, ci:ci + 1], None,
                                        op0=Alu.mult)
            mx = sc.tile([128, 1], f32, tag="sc")
            nc.vector.tensor_reduce(mx[:], gcoll[:].rearrange("p a b -> p (a b)"),
                                    axis=mybir.AxisListType.X, op=Alu.max)
            amax_all = sc.tile([128, 1], f32, tag="sc")
            nc.gpsimd.partition_all_reduce(amax_all[:], mx[:], channels=128,
                                           reduce_op=bass_isa.ReduceOp.max)
            nc.sync.dma_start(cc_in[:], amax_all[:])
            if n_cores > 1:
                nc.gpsimd.collective_compute(
                    "AllReduce", Alu.max,
                    replica_groups=[list(range(n_cores))],
                    ins=[cc_in[:].opt()], outs=[cc_out[:].opt()])
            else:
                nc.sync.dma_start(cc_out[:], cc_in[:])

            # ---- wscale from wsqs (runs during the collective) ----
            wsacc = sc.tile([128, 1], f32, tag="sc")
            nc.vector.tensor_reduce(wsacc[:], wsqs[:],
                                    axis=mybir.AxisListType.X, op=Alu.add)
            wsum_ps = ps_w.tile([1, 1], f32, tag="wsum")
            nc.tensor.matmul(wsum_ps[:], wsacc[:], ones_col[:, 0:1],
                             start=True, stop=True)
            wscale = sc.tile([1, 1], f32, tag="sc")
            nc.scalar.copy(wscale[:], wsum_ps[:])
            nc.vector.tensor_scalar(wscale[:], wscale[:], 1.0 / W_COUNT,
                                    EPS_SCALE, op0=Alu.mult, op1=Alu.max)
            winv = sc.tile([1, 1], f32, tag="sc")
            nc.vector.reciprocal(winv[:], wscale[:])
            winv_col = sc.tile([128, 1], f32, tag="sc")
            nc.gpsimd.partition_broadcast(winv_col[:], winv[:])
            ws_dram = dramp.tile([1], f32)
            nc.sync.dma_start(ws_dram[:].rearrange("(a d) -> a d", a=1),
                              wscale[:])

            # ---------------- phase 2: quantize + conv ----------------
            qhp = stk.enter_context(tc.tile_pool(name="qhp", bufs=1))
            qlp = stk.enter_context(tc.tile_pool(name="qlp", bufs=1))
            xinb = stk.enter_context(tc.tile_pool(name="xinb", bufs=4))
            tscr = stk.enter_context(tc.tile_pool(name="tscr", bufs=2))
            outp = stk.enter_context(tc.tile_pool(name="outp", bufs=3))

            qh_sb = qhp.tile([128, CI, t_len], fp8)
            ql_sb = qlp.tile([128, CI, t_len], fp8)
            wq8 = wqp.tile([128, WQ_F], fp8)
            wqv = wq8[:].rearrange("p (cb k ci o) -> p cb k ci o",
                                   cb=CB, k=K, ci=CI)

            xg2_tiles = {}

            def emit_xdma(gq):
                xg2 = xinb.tile([128, CI, QG], f32, tag="xin2")
                nc.sync.dma_start(xg2[:], xv[:, :, ts(gq, QG)])
                xg2_tiles[gq] = xg2

            for gq in range(4):
                emit_xdma(gq)
            # pass-B DMAs dispatch behind the prefetch; wstb bufs throttle
            for e in range(1, NWB):
                wb_tiles[e] = passB_dma(e)
            # Pool computes chunks 0-5 (it is otherwise idle from here on)
            for e in range(6):
                passB_compute(e, wb_tiles[e], nc.gpsimd)

            # ---- per-partition scale columns via DMA broadcast ----
            v_col = sc.tile([128, 1], f32, tag="sc")
            nc.sync.dma_start(
                v_col[:],
                cc_out[0:1].rearrange("(a d) -> a d", a=1)
                .partition_broadcast(128))
            ws_col = sc.tile([128, 1], f32, tag="sc")
            nc.sync.dma_start(
                ws_col[:],
                ws_dram[:].rearrange("(a d) -> a d", a=1)
                .partition_broadcast(128))

            def passB_dma(e):
                w8 = wsta.tile([128, WB], f32, tag="wstb")
                nc.sync.dma_start(w8[:], wt_t[:, ts(e, WB)])
                return w8

            def passB_compute(e, w8, eng):
                eng.tensor_scalar(w8[:], w8[:], winv_col[:], C1,
                                  op0=Alu.mult, op1=Alu.add)
                eng.tensor_scalar(w8[:], w8[:], C1 + 1.0, C1 - 1.0,
                                  op0=Alu.min, op1=Alu.max)
                eng.tensor_scalar(wq8[:, ts(e, WB)], w8[:], C1, None,
                                  op0=Alu.subtract)

            wb_tiles = {0: passB_dma(0)}

            # ---- post-AllReduce scale math on [128,1] columns (DVE) ----
            qs_col = sc.tile([128, 1], f32, tag="sc")
            nc.vector.tensor_scalar_max(qs_col[:], v_col[:], EPS_SCALE)
            qi_col = sc.tile([128, 1], f32, tag="sc")
            nc.vector.reciprocal(qi_col[:], qs_col[:])
            q254_col = sc.tile([128, 1], f32, tag="sc")
            nc.vector.tensor_scalar_mul(q254_col[:], qi_col[:], 2.0 * QP)
            gq_cm = sc.tile([128, CI], f32, tag="gq")
            nc.vector.tensor_scalar(gq_cm[:], g_cm[:], q254_col[:], None,
                                    op0=Alu.mult)
            fs_col = sc.tile([128, 1], f32, tag="sc")
            nc.vector.tensor_tensor(fs_col[:], ws_col[:], qs_col[:],
                                    op=Alu.mult)
            nc.vector.tensor_scalar_mul(fs_col[:], fs_col[:], 1.0 / QP)


            def quant_grain(gq):
                xg2 = xg2_tiles.pop(gq)
                for ci in range(CI):
                    u2 = tscr.tile([128, QG], f32, tag="u2")
                    nc.vector.tensor_tensor(u2[:], xg2[:, ci, :],
                                            R[:, ts(gq, QG)], op=Alu.mult)
                    t1 = tscr.tile([128, QG], f32, tag="t1")
                    nc.scalar.activation(t1[:], u2[:], Act.Identity,
                                         bias=c1_col[:],
                                         scale=gq_cm[:, ci:ci + 1])
                    t3 = tscr.tile([128, QG], f32, tag="t3")
                    nc.vector.tensor_scalar(t3[:], t1[:], D8, D8,
                                            op0=Alu.add, op1=Alu.subtract)
                    nc.scalar.activation(qh_sb[:, ci, ts(gq, QG)], t3[:],
                                         Act.Identity, bias=mc1_col[:],
                                         scale=1.0)
                    nc.vector.tensor_tensor(ql_sb[:, ci, ts(gq, QG)], t1[:],
                                            t3[:], op=Alu.subtract)

            def conv_tile(cb, j):
                cps = ps_conv.tile([128, TT], f32, tag="conv")
                n_mm = 0
                for k in (3, 0, 1, 2, 4, 5, 6):
                    lo_data = j * TT + k - PAD
                    out_lo = max(0, -lo_data)
                    out_hi = TT - max(0, lo_data + TT - t_len)
                    a, b = lo_data + out_lo, lo_data + out_hi
                    for src in (qh_sb, ql_sb):
                        for cp in range(2):
                            nc.tensor.matmul(
                                cps[:, out_lo:out_hi],
                                wqv[:, cb, k, 2 * cp:2 * cp + 2, :],
                                src[:, 2 * cp:2 * cp + 2, a:b],
                                perf_mode=DR,
                                start=(n_mm == 0), stop=(n_mm == 27))
                            n_mm += 1
                osb = outp.tile([128, TT], f32)
                nc.scalar.activation(osb[:], cps[:], Act.Copy,
                                     scale=fs_col[:])
                nc.sync.dma_start(out_t[ts(cb, 128), ts(j, TT)], osb[:])

            # software pipeline: quant stays one group ahead of conv;
            # remaining even pass-B chunks slot in between quant groups
            nqg = t_len // QG
            QPG = GT // QG  # quant grains per group
            for G in range(ng):
                for gq in range(G * QPG, (G + 1) * QPG):
                    if gq + 4 < nqg:
                        emit_xdma(gq + 4)
                    quant_grain(gq)
                if G in (0, 1):
                    passB_compute(6 + G, wb_tiles[6 + G], nc.vector)
                if G >= 1:
                    for cb in range(CB):
                        for j in range((G - 1) * GRP, G * GRP):
                            conv_tile(cb, j)
            for cb in range(CB):
                for j in range((ng - 1) * GRP, nt):
                    conv_tile(cb, j)

    nc.compile()
    return nc


def _prep_weight(weight: np.ndarray) -> np.ndarray:
    # WT[p, cb, k, ci, o'] = weight[cb*128+o', ci*128+p, k], flattened to
    # (128, 14336) so lhsT tiles are contiguous slices.
    w = np.ascontiguousarray(weight.astype(np.float32, copy=False))
    w5 = w.reshape(CB, 128, CI, 128, K)  # [cb, o', ci, p, k]
    wt = w5.transpose(3, 0, 4, 2, 1)  # [p, cb, k, ci, o']
    return np.ascontiguousarray(wt.reshape(128, -1))


def kernel(x: np.ndarray, weight: np.ndarray, gamma: np.ndarray) -> np.ndarray:
    from concourse.bass_utils import run_bass_kernel_spmd

    key = ("full", N_CORES, T)
    if key not in _CACHE:
        _CACHE[key] = _build(N_CORES, T)
    nc = _CACHE[key]

    wt = _prep_weight(weight)
    g = np.ascontiguousarray(gamma.astype(np.float32, copy=False))
    in_maps = [
        {"x": np.ascontiguousarray(x[b].astype(np.float32, copy=False)),
         "wt": wt, "g": g}
        for b in range(N_CORES)
    ]
    res = run_bass_kernel_spmd(nc, in_maps, list(range(N_CORES)))
    out = np.stack([res.results[b]["out"] for b in range(N_CORES)], axis=0)
    return out


# revision 30
# speedup vs baseline: 1.7907x; 1.0068x over previous
"""BitConv1d Trainium2 kernel (fp8 DoubleRow version).

Computes, for x:(8,512,8192) f32, weight:(512,512,7) f32, gamma:(512,) f32:
  rms  = sqrt(mean(x^2, channel) + 1e-6)          (per b,t)
  xn   = x / rms * gamma
  s    = max(|xn|) over the FULL batch  (clamped to >= 1e-5)
  q    = round(clip(xn/s*127, -128, 127))         (8-bit act quant)
  ws   = max(mean(|w|), 1e-5); wq = round(clip(w/ws, -1, 1))  (ternary)
  out  = conv1d(q * s/127, wq, pad 3) * ws

Strategy: data-parallel over batch across 8 NeuronCores (1 batch element
per core), weights replicated; AllReduce(max) for the global activation
scale. The conv runs as fp8e4 DoubleRow matmuls with an exact integer
split q = qh + ql (qh = round8(q), a multiple of 8 with |qh| <= 128;
ql = q - qh in [-4,4]); these values and the ternary weights are exactly
representable in fp8e4, and f32 PSUM accumulation of integers < 2^24 is
exact, so the conv equals the integer conv, scaled afterward. Rounding
uses (x + 1.5*2^23) - 1.5*2^23 (round-half-even) and the
round-to-multiple-of-8 variant with 1.5*2^26.
"""

import sys

sys.path.insert(0, "/opt/trn_rl_repo")

import numpy as np

N_CORES = 8
B, C, T = 8, 512, 8192
CO, K = 512, 7
CI = 4          # in-channel chunks of 128
CB = 4          # out-channel blocks of 128
TT = 512        # conv time-tile (psum bank width)
PAD = 3
GRP = 4         # j-tiles per phase-1 group
GT = GRP * TT   # 2048 cols per group
QG = 512        # phase-2 quant grain

EPS_NORM = 1e-6
EPS_SCALE = 1e-5
QP = 127.0
C1 = 12582912.0          # 1.5*2^23: round to int
D8 = 7.5 * 2 ** 23       # C8-C1 where C8 = 1.5*2^26: round to multiple of 8
W_COUNT = CO * C * K
WQ_F = CB * K * CI * 128  # 14336 weight columns
NWA = 16                  # pass-A chunks
NWB = 8                   # pass-B chunks

_CACHE = {}


def _build(n_cores: int, t_len: int):
    import contextlib

    import concourse.bacc as bacc
    import concourse.bass as bass
    import concourse.tile as tile
    from concourse import bass_isa, mybir

    f32 = mybir.dt.float32
    fp8 = mybir.dt.float8e4
    Alu = mybir.AluOpType
    Act = mybir.ActivationFunctionType
    DR = mybir.MatmulPerfMode.DoubleRow
    ts = bass.ts

    nt = t_len // TT
    ng = nt // GRP
    nq = t_len // QG
    WA = WQ_F // NWA  # 896
    WB = WQ_F // NWB  # 1792

    nc = bacc.Bacc("TRN2", target_bir_lowering=False, debug=False,
                   num_devices=n_cores)

    x_t = nc.dram_tensor("x", [C, t_len], f32, kind="ExternalInput")
    wt_t = nc.dram_tensor("wt", [128, WQ_F], f32, kind="ExternalInput")
    g_t = nc.dram_tensor("g", [C], f32, kind="ExternalInput")
    out_t = nc.dram_tensor("out", [CO, t_len], f32, kind="ExternalOutput")

    xv = x_t[:].rearrange("(c p) t -> p c t", p=128)  # chunk-major channels

    with tile.TileContext(nc) as tc:
        with contextlib.ExitStack() as stk:
            singles = stk.enter_context(tc.tile_pool(name="singles", bufs=1))
            sc = stk.enter_context(tc.tile_pool(name="sc", bufs=24))
            rlinp = stk.enter_context(tc.tile_pool(name="rlin", bufs=1))
            wqp = stk.enter_context(tc.tile_pool(name="wqp", bufs=1))
            wsta = stk.enter_context(tc.tile_pool(name="wsta", bufs=2))
            dramp = stk.enter_context(
                tc.tile_pool(name="dram", bufs=1, space="DRAM"))
            drbp = stk.enter_context(
                tc.tile_pool(name="drb", bufs=4, space="DRAM"))
            ps_ssq = stk.enter_context(
                tc.tile_pool(name="ps_ssq", bufs=2, space="PSUM"))
            ps_w = stk.enter_context(
                tc.tile_pool(name="ps_w", bufs=1, space="PSUM"))
            ps_conv = stk.enter_context(
                tc.tile_pool(name="ps_conv", bufs=4, space="PSUM"))

            ones_col = singles.tile([128, 1], f32)
            nc.vector.memset(ones_col[:], 1.0)
            eps_col = singles.tile([128, 1], f32)
            nc.vector.memset(eps_col[:], EPS_NORM)
            c1_col = singles.tile([128, 1], f32)
            nc.vector.memset(c1_col[:], C1)
            mc1_col = singles.tile([128, 1], f32)
            nc.vector.memset(mc1_col[:], -C1)
            zmov = singles.tile([128, GRP * 4], f32)
            nc.vector.memset(zmov[:], 0.0)
            g_cm = singles.tile([128, CI], f32)  # gamma, chunk-major
            nc.sync.dma_start(g_cm[:], g_t[:].rearrange("(c p) -> p c", p=128))
            gabs_cm = singles.tile([128, CI], f32)  # 2*|gamma|
            nc.scalar.activation(gabs_cm[:], g_cm[:], Act.Abs, scale=2.0)

            cc_in = dramp.tile([128], f32)
            cc_out = dramp.tile([128], f32)

            # R = rhalf broadcast to all partitions; persists into phase 2
            R = rlinp.tile([128, t_len], f32)
            coll = singles.tile([128, CI, nt], f32)

            # ---------------- phase 1: rms + local max ----------------
            # j-tile granular pipeline: per 512-col tile, the serial chain
            # dma -> square -> ssq colsums -> rms -> bounce -> broadcast ->
            # v=x*R -> reduce has ~6us latency, hidden by 6 x-tile buffers.
            with tc.tile_pool(name="xina", bufs=6) as xinp, \
                 tc.tile_pool(name="x2a", bufs=3) as x2p, \
                 tc.tile_pool(name="rms", bufs=4) as rmsp, \
                 tc.tile_pool(name="vs", bufs=4) as vsp:
                vq = []

                def emit_vred(jj, xg):
                    for ci in range(CI):
                        v = vsp.tile([128, TT], f32, tag="v")
                        eng = nc.vector if ci < 2 else nc.gpsimd
                        eng.tensor_tensor(v[:], xg[:, ci, :],
                                          R[:, ts(jj, TT)], op=Alu.mult)
                        nc.vector.tensor_reduce(coll[:, ci, jj:jj + 1], v[:],
                                                axis=mybir.AxisListType.X,
                                                op=Alu.max,
                                                apply_absolute_value=True)

                for jj in range(nt):
                    xg = xinp.tile([128, CI, TT], f32, tag="xin")
                    nc.sync.dma_start(xg[:], xv[:, :, ts(jj, TT)])
                    x2 = x2p.tile([128, CI, TT], f32, tag="x2")
                    nc.scalar.activation(x2[:], xg[:], Act.Square)
                    sp = ps_ssq.tile([128, 4], f32, tag="ssq")
                    nc.tensor.matmul(sp[:], x2[:, 0, 0:128], zmov[:, 0:4],
                                     start=True, stop=False)
                    for b in range(4):  # 4 t-blocks per j-tile
                        for ci in range(CI):
                            nc.tensor.matmul(
                                sp[:, b:b + 1],
                                x2[:, ci, ts(b, 128)], ones_col[:],
                                start=False,
                                stop=(b == 3 and ci == CI - 1))
                    rc = rmsp.tile([128, 4], f32, tag="rc")
                    nc.scalar.copy(rc[:], sp[:])
                    # rms math for this tile's 4 blocks (Newton refined)
                    mc = rmsp.tile([128, 4], f32, tag="mc")
                    nc.vector.tensor_scalar(mc[:], rc[:], 1.0 / C,
                                            EPS_NORM, op0=Alu.mult,
                                            op1=Alu.add)
                    s0 = rmsp.tile([128, 4], f32, tag="s0")
                    nc.scalar.activation(s0[:], rc[:], Act.Sqrt,
                                         bias=eps_col[:], scale=1.0 / C)
                    td = rmsp.tile([128, 4], f32, tag="td")
                    nc.vector.reciprocal(td[:], s0[:])
                    nc.vector.tensor_tensor(td[:], mc[:], td[:], op=Alu.mult)
                    nc.vector.tensor_tensor(td[:], td[:], s0[:], op=Alu.add)
                    rh = rmsp.tile([128, 4], f32, tag="rh")
                    nc.vector.reciprocal(rh[:], td[:])
                    # bounce rhalf to DRAM, broadcast back to all rows
                    rb = drbp.tile([TT], f32, tag="rb")
                    nc.scalar.dma_start(
                        rb[:].rearrange("(b p) -> p b", p=128), rh[:])
                    nc.gpsimd.dma_start(
                        R[:, ts(jj, TT)],
                        rb[:].rearrange("(a d) -> a d", a=1)
                        .partition_broadcast(128))
                    # local max |x*rhalf| per (ci, tile), emitted two
                    # tiles late so the R broadcast has already landed and
                    # DVE's in-order stream never waits on it
                    vq.append((jj, xg))
                    if len(vq) > 2:
                        emit_vred(*vq.pop(0))
                for jj, xg in vq:
                    emit_vred(jj, xg)

            # ---- weight pass A: DMAs early (transfers overlap collective);
            # each chunk's |w| column-sum lands in its own wsqs column ----
            wsqs = sc.tile([128, NWA], f32, tag="wsqs")
            for e in range(NWA):
                wt_e = wsta.tile([128, WB], f32, tag="wstb")
                nc.sync.dma_start(wt_e[:, 0:WA], wt_t[:, ts(e, WA)])
                nc.scalar.activation(wt_e[:, 0:WA], wt_e[:, 0:WA], Act.Abs,
                                     accum_out=wsqs[:, e:e + 1])

            # ---- global max + AllReduce ----
            gcoll = sc.tile([128, CI, nt], f32, tag="gcoll")
            for ci in range(CI):
                nc.vector.tensor_scalar(gcoll[:, ci, :], coll[:, ci, :],
                                        gabs_cm[:, ci:ci + 1], None,
                                        op0=Alu.mult)
            mx = sc.tile([128, 1], f32, tag="sc")
            nc.vector.tensor_reduce(mx[:], gcoll[:].rearrange("p a b -> p (a b)"),
                                    axis=mybir.AxisListType.X, op=Alu.max)
            amax_all = sc.tile([128, 1], f32, tag="sc")
            nc.gpsimd.partition_all_reduce(amax_all[:], mx[:], channels=128,
                                           reduce_op=bass_isa.ReduceOp.max)
            nc.sync.dma_start(cc_in[:], amax_all[:])
            if n_cores > 1:
                nc.gpsimd.collective_compute(
                    "AllReduce", Alu.max,
                    replica_groups=[list(range(n_cores))],
                    ins=[cc_in[:].opt()], outs=[cc_out[:].opt()])
            else:
                nc.sync.dma_start(cc_out[:], cc_in[:])

            # ---- wscale from wsqs (runs during the collective) ----
            wsacc = sc.tile([128, 1], f32, tag="sc")
            nc.vector.tensor_reduce(wsacc[:], wsqs[:],
                                    axis=mybir.AxisListType.X, op=Alu.add)
            wsum_ps = ps_w.tile([1, 1], f32, tag="wsum")
            nc.tensor.matmul(wsum_ps[:], wsacc[:], ones_col[:, 0:1],
                             start=True, stop=True)
            wscale = sc.tile([1, 1], f32, tag="sc")
            nc.scalar.copy(wscale[:], wsum_ps[:])
            nc.vector.tensor_scalar(wscale[:], wscale[:], 1.0 / W_COUNT,
                                    EPS_SCALE, op0=Alu.mult, op1=Alu.max)
            winv = sc.tile([1, 1], f32, tag="sc")
            nc.vector.reciprocal(winv[:], wscale[:])
            winv_col = sc.tile([128, 1], f32, tag="sc")
            nc.gpsimd.partition_broadcast(winv_col[:], winv[:])
            ws_dram = dramp.tile([1], f32)
            nc.sync.dma_start(ws_dram[:].rearrange("(a d) -> a d", a=1),
                              wscale[:])

            # ---------------- phase 2: quantize + conv ----------------
            qhp = stk.enter_context(tc.tile_pool(name="qhp", bufs=1))
            qlp = stk.enter_context(tc.tile_pool(name="qlp", bufs=1))
            xinb = stk.enter_context(tc.tile_pool(name="xinb", bufs=4))
            tscr = stk.enter_context(tc.tile_pool(name="tscr", bufs=2))
            outp = stk.enter_context(tc.tile_pool(name="outp", bufs=3))

            qh_sb = qhp.tile([128, CI, t_len], fp8)
            ql_sb = qlp.tile([128, CI, t_len], fp8)
            wq8 = wqp.tile([128, WQ_F], fp8)
            wqv = wq8[:].rearrange("p (cb k ci o) -> p cb k ci o",
                                   cb=CB, k=K, ci=CI)

            xg2_tiles = {}

            def emit_xdma(gq):
                xg2 = xinb.tile([128, CI, QG], f32, tag="xin2")
                nc.sync.dma_start(xg2[:], xv[:, :, ts(gq, QG)])
                xg2_tiles[gq] = xg2

            # interleave pass-B DMAs 1:1 with the xg2 prefetch so the
            # first weight chunks land early on the serial DMA stream
            for gq in range(4):
                emit_xdma(gq)
                wb_tiles[2 * gq + 1] = passB_dma(2 * gq + 1)
                if 2 * gq + 2 < NWB:
                    wb_tiles[2 * gq + 2] = passB_dma(2 * gq + 2)
            # Pool computes chunks 0-5 (it is otherwise idle from here on)
            for e in range(6):
                passB_compute(e, wb_tiles[e], nc.gpsimd)

            # ---- per-partition scale columns via DMA broadcast ----
            v_col = sc.tile([128, 1], f32, tag="sc")
            nc.sync.dma_start(
                v_col[:],
                cc_out[0:1].rearrange("(a d) -> a d", a=1)
                .partition_broadcast(128))
            ws_col = sc.tile([128, 1], f32, tag="sc")
            nc.sync.dma_start(
                ws_col[:],
                ws_dram[:].rearrange("(a d) -> a d", a=1)
                .partition_broadcast(128))

            def passB_dma(e):
                w8 = wsta.tile([128, WB], f32, tag="wstb")
                nc.sync.dma_start(w8[:], wt_t[:, ts(e, WB)])
                return w8

            def passB_compute(e, w8, eng):
                eng.tensor_scalar(w8[:], w8[:], winv_col[:], C1,
                                  op0=Alu.mult, op1=Alu.add)
                eng.tensor_scalar(w8[:], w8[:], C1 + 1.0, C1 - 1.0,
                                  op0=Alu.min, op1=Alu.max)
                eng.tensor_scalar(wq8[:, ts(e, WB)], w8[:], C1, None,
                                  op0=Alu.subtract)

            wb_tiles = {0: passB_dma(0)}

            # ---- post-AllReduce scale math on [128,1] columns (DVE) ----
            qs_col = sc.tile([128, 1], f32, tag="sc")
            nc.vector.tensor_scalar_max(qs_col[:], v_col[:], EPS_SCALE)
            qi_col = sc.tile([128, 1], f32, tag="sc")
            nc.vector.reciprocal(qi_col[:], qs_col[:])
            q254_col = sc.tile([128, 1], f32, tag="sc")
            nc.vector.tensor_scalar_mul(q254_col[:], qi_col[:], 2.0 * QP)
            gq_cm = sc.tile([128, CI], f32, tag="gq")
            nc.vector.tensor_scalar(gq_cm[:], g_cm[:], q254_col[:], None,
                                    op0=Alu.mult)
            fs_col = sc.tile([128, 1], f32, tag="sc")
            nc.vector.tensor_tensor(fs_col[:], ws_col[:], qs_col[:],
                                    op=Alu.mult)
            nc.vector.tensor_scalar_mul(fs_col[:], fs_col[:], 1.0 / QP)


            def quant_grain(gq):
                xg2 = xg2_tiles.pop(gq)
                for ci in range(CI):
                    u2 = tscr.tile([128, QG], f32, tag="u2")
                    nc.vector.tensor_tensor(u2[:], xg2[:, ci, :],
                                            R[:, ts(gq, QG)], op=Alu.mult)
                    t1 = tscr.tile([128, QG], f32, tag="t1")
                    nc.scalar.activation(t1[:], u2[:], Act.Identity,
                                         bias=c1_col[:],
                                         scale=gq_cm[:, ci:ci + 1])
                    t3 = tscr.tile([128, QG], f32, tag="t3")
                    nc.vector.tensor_scalar(t3[:], t1[:], D8, D8,
                                            op0=Alu.add, op1=Alu.subtract)
                    nc.scalar.activation(qh_sb[:, ci, ts(gq, QG)], t3[:],
                                         Act.Identity, bias=mc1_col[:],
                                         scale=1.0)
                    nc.vector.tensor_tensor(ql_sb[:, ci, ts(gq, QG)], t1[:],
                                            t3[:], op=Alu.subtract)

            def conv_tile(cb, j):
                cps = ps_conv.tile([128, TT], f32, tag="conv")
                n_mm = 0
                for k in (3, 0, 1, 2, 4, 5, 6):
                    lo_data = j * TT + k - PAD
                    out_lo = max(0, -lo_data)
                    out_hi = TT - max(0, lo_data + TT - t_len)
                    a, b = lo_data + out_lo, lo_data + out_hi
                    for src in (qh_sb, ql_sb):
                        for cp in range(2):
                            nc.tensor.matmul(
                                cps[:, out_lo:out_hi],
                                wqv[:, cb, k, 2 * cp:2 * cp + 2, :],
                                src[:, 2 * cp:2 * cp + 2, a:b],
                                perf_mode=DR,
                                start=(n_mm == 0), stop=(n_mm == 27))
                            n_mm += 1
                osb = outp.tile([128, TT], f32)
                nc.scalar.activation(osb[:], cps[:], Act.Copy,
                                     scale=fs_col[:])
                nc.sync.dma_start(out_t[ts(cb, 128), ts(j, TT)], osb[:])

            # software pipeline: quant stays one group ahead of conv;
            # remaining even pass-B chunks slot in between quant groups
            nqg = t_len // QG
            QPG = GT // QG  # quant grains per group
            for G in range(ng):
                for gq in range(G * QPG, (G + 1) * QPG):
                    if gq + 4 < nqg:
                        emit_xdma(gq + 4)
                    quant_grain(gq)
                if G in (0, 1):
                    passB_compute(6 + G, wb_tiles[6 + G], nc.vector)
                if G >= 1:
                    for cb in range(CB):
                        for j in range((G - 1) * GRP, G * GRP):
                            conv_tile(cb, j)
            for cb in range(CB):
                for j in range((ng - 1) * GRP, nt):
                    conv_tile(cb, j)

    nc.compile()
    return nc


def _prep_weight(weight: np.ndarray) -> np.ndarray:
    # WT[p, cb, k, ci, o'] = weight[cb*128+o', ci*128+p, k], flattened to
    # (128, 14336) so lhsT tiles are contiguous slices.
    w = np.ascontiguousarray(weight.astype(np.float32, copy=False))
    w5 = w.reshape(CB, 128, CI, 128, K)  # [cb, o', ci, p, k]
    wt = w5.transpose(3, 0, 4, 2, 1)  # [p, cb, k, ci, o']
    return np.ascontiguousarray(wt.reshape(128, -1))


def kernel(x: np.ndarray, weight: np.ndarray, gamma: np.ndarray) -> np.ndarray:
    from concourse.bass_utils import run_bass_kernel_spmd

    key = ("full", N_CORES, T)
    if key not in _CACHE:
        _CACHE[key] = _build(N_CORES, T)
    nc = _CACHE[key]

    wt = _prep_weight(weight)
    g = np.ascontiguousarray(gamma.astype(np.float32, copy=False))
    in_maps = [
        {"x": np.ascontiguousarray(x[b].astype(np.float32, copy=False)),
         "wt": wt, "g": g}
        for b in range(N_CORES)
    ]
    res = run_bass_kernel_spmd(nc, in_maps, list(range(N_CORES)))
    out = np.stack([res.results[b]["out"] for b in range(N_CORES)], axis=0)
    return out
